# revision 9
# baseline (speedup 1.0000x reference)
"""Trainium2 Bass kernel for nn_Dsa_Decoder.

Math note (why this kernel is small): in the reference,
``beta = log_softmax(score, axis=-1)`` is taken over a singleton axis, so
``beta`` is exactly 0 and the context vector ``ctx2 = einsum(beta, enc_h)``
is exactly zero at every step. Each step's LSTM input is therefore
``x = d_t * dense_w[0,0] + dense_b`` (the ctx part of the dense layer
contributes exactly +0.0), and the LSTM always restarts from (h0, c0), so
step outputs are independent across time: the scan's final carry is just
the last step's ``h_s`` plus a zero context. The full module collapses to
one LSTM cell evaluated at ``d = t[:, -1]``:

    gates = [h0 | x | 1] @ [w_hh.T ; w_ih.T ; (b_ih+b_hh)]      (B, 4H)
    c2 = sigmoid(f) * c0 + sigmoid(i) * tanh(g)
    h2 = sigmoid(o) * tanh(c2)
    out = concat([h2, zeros], -1)                               (B, 1, 2H)

Sharding: pure data parallel — batch 512 split across 8 cores (64 rows
each); the tiny weights are replicated. enc_h and the attention weights
never reach the device (they only feed the exactly-zero branch).

Implementation: raw Bass (no TileContext) with hand-placed semaphores, to
avoid Tile's end-of-kernel drain + double all-engine barrier (~9us on a
~14us kernel). All device inputs are packed into ONE dram tensor so there
is a single input DMA. Semaphores are cleared by their last waiter so the
program is safely re-executable without any end barrier.

Per-core device program:
  sync: dma(packed->SBUF); wait v>=4; dma(h2->DRAM); wait d>=32; clear d,v
  PE:   wait d>=16; matmul gates(64x256) = packed[:, :64].T @ packed[:, 64:320]
  ACT:  [table preload]; wait p>=1; tanh(g); sigmoid(i|f|o); clear p;
        wait v>=3; tanh(c2)
  DVE:  wait a>=2; i*tanh_g; f*c0; add -> c2; wait a>=3; clear a; o*tanh_c2
Gate columns are pre-permuted to [i | f | o | g] so the three sigmoids are
a single ACT instruction.
"""

import numpy as np

import concourse.bass as bass
import concourse.bacc as bacc
import concourse.mybir as mybir
from concourse import bass_utils

B, T, H = 512, 64, 64
N_CORES = 8
BP = B // N_CORES          # 64 batch rows per core
K = H + 2                  # contraction dim: 64 h + 1 x + 1 bias row
G4 = 4 * H                 # 256 gate columns
PACK_W = H + G4 + H        # 384: [aT | w | c0]

_NC_CACHE = None


def _build_nc(sem_clears=True, detect_races=False):
    """Build + compile the per-core Bass program (cached across calls).

    sem_clears=True restores all semaphores to 0 at the end of the
    program so the NEFF is safely re-executable. The clears are placed on
    each semaphore's final observer (safe: executions serialize at NEFF
    boundaries), which the CoreSim race checker can't prove — so race
    validation (sim_check.py) uses a sem_clears=False build and numerics
    use this one with the checker off.
    """
    global _NC_CACHE
    if _NC_CACHE is not None and sem_clears and not detect_races:
        return _NC_CACHE

    nc = bacc.Bacc("TRN2", target_bir_lowering=False, debug=False,
                   num_devices=N_CORES, detect_race_conditions=detect_races)
    f32 = mybir.dt.float32
    AF = mybir.ActivationFunctionType
    packed_d = nc.dram_tensor("packed", (K, PACK_W), f32, kind="ExternalInput")
    h2_d = nc.dram_tensor("h2", (BP, H), f32, kind="ExternalOutput")

    with (
        nc.sbuf_tensor("sb", [K, PACK_W], f32) as sb,
        nc.sbuf_tensor("sig", [BP, 3 * H], f32) as sig,
        nc.sbuf_tensor("tg", [BP, H], f32) as tg,
        nc.sbuf_tensor("t1", [BP, H], f32) as t1,
        nc.sbuf_tensor("t2", [BP, H], f32) as t2,
        nc.sbuf_tensor("c2", [BP, H], f32) as c2,
        nc.sbuf_tensor("tc2", [BP, H], f32) as tc2,
        nc.sbuf_tensor("h2_sb", [BP, H], f32) as h2,
        nc.sbuf_tensor("scratch", [BP, 1], f32) as scratch,
        nc.psum_tensor("gates", [BP, G4], f32) as gates,
        nc.semaphore("d_in") as d_in,
        nc.semaphore("d_out") as d_out,
        nc.semaphore("p") as p,
        nc.semaphore("a") as a,
        nc.semaphore("v") as v,
    ):
        sy, pe, act, dve = nc.sync, nc.tensor, nc.scalar, nc.vector

        # sync: the two DMAs. Sem clears are placed after a later
        # instruction so the pending wait_ge nop-fuses onto a non-clear
        # instruction (the race checker requires updates to be consumed
        # by a wait that precedes the clear).
        sy.dma_start(sb[:], packed_d[:]).then_inc(d_in, 16)
        sy.wait_ge(v, 4)
        sy.dma_start(h2_d[:], h2[:]).then_inc(d_out, 16)
        sy.wait_ge(d_out, 16)
        if sem_clears:
            sy.sem_clear(v)
            sy.sem_clear(d_out)

        # PE: single matmul, contraction over K=66. Instructions may lower
        # to several ISA chunks, each of which re-fires a then_inc — so all
        # compute-completion signaling below uses explicit drain + sem_inc,
        # which is chunk-count independent.
        pe.wait_ge(d_in, 16)
        pe.matmul(gates[:], sb[:, 0:H], sb[:, H:H + G4], start=True, stop=True)
        pe.drain()
        if sem_clears:
            pe.sem_clear(d_in)
        pe.sem_inc(p, 1)

        # ACT: dummy activation first so Bacc's table-load pass puts the
        # ACT_TABLE_LOAD at program start (overlapping the DMA + matmul)
        # instead of behind the wait on the matmul.
        act.memzero(scratch[:])
        act.drain()
        act.activation(scratch[:], scratch[:], AF.Sigmoid)
        act.wait_ge(p, 1)
        act.activation(tg[:], gates[:, 3 * H:G4], AF.Tanh)
        act.activation(sig[:], gates[:, 0:3 * H], AF.Sigmoid)
        act.drain()
        if sem_clears:
            act.sem_clear(p)
        act.sem_inc(a, 1)
        act.wait_ge(v, 3)
        act.activation(tc2[:], c2[:], AF.Tanh)
        act.drain()
        act.sem_inc(a, 1)

        # DVE: gate combine
        dve.wait_ge(a, 1)
        dve.tensor_mul(t2[:], sig[:, 0:H], tg[:])                      # i*tanh(g)
        dve.tensor_mul(t1[:], sig[:, H:2 * H],
                       sb[0:BP, H + G4:PACK_W])                        # f*c0
        dve.drain()                # DVE is pipelined: RAW on t1/t2 needs sync
        dve.tensor_add(c2[:], t1[:], t2[:])
        dve.drain()
        dve.sem_inc(v, 3)
        dve.wait_ge(a, 2)
        dve.tensor_mul(h2[:], sig[:, 2 * H:3 * H], tc2[:])
        dve.drain()
        if sem_clears:
            dve.sem_clear(a)
        dve.sem_inc(v, 1)

    nc.compile()
    if sem_clears and not detect_races:
        _NC_CACHE = nc
    return nc


def _pack_inputs(t, h0, c0, dense_w, dense_b, w_ih, w_hh, b_ih, b_hh):
    """Host-side shard + layout packing (tiny: O(B*H + H^2) floats)."""
    d = t[:, -1]                                    # (B,) last time step
    x = d * dense_w[0, 0] + dense_b[0]              # (B,) dense layer on [d, 0ctx]

    # Gate columns permuted to [i | f | o | g].
    perm = np.concatenate([np.arange(0, H), np.arange(H, 2 * H),
                           np.arange(3 * H, 4 * H), np.arange(2 * H, 3 * H)])
    w = np.empty((K, G4), np.float32)
    w[:H] = w_hh.T[:, perm]
    w[H] = w_ih[perm, 0]
    w[H + 1] = (b_ih + b_hh)[perm]

    h = h0[0]                                       # (B, H)
    c = c0[0]                                       # (B, H)
    in_maps = []
    for core in range(N_CORES):
        r = slice(core * BP, (core + 1) * BP)
        packed = np.zeros((K, PACK_W), np.float32)
        packed[:H, 0:H] = h[r].T                    # aT rows 0:64
        packed[H, 0:H] = x[r]                       # x row
        packed[H + 1, 0:H] = 1.0                    # ones row
        packed[:, H:H + G4] = w
        packed[0:BP, H + G4:PACK_W] = c[r]          # c0 block
        in_maps.append({"packed": packed})
    return in_maps


def kernel(t, enc_h, h0, c0, dense_w, dense_b, w_ih, w_hh, b_ih, b_hh,
           w1_w, w1_b, w2_w, w2_b, v_w, v_b, **_unused):
    t = np.asarray(t, np.float32)
    h0 = np.asarray(h0, np.float32)
    c0 = np.asarray(c0, np.float32)
    dense_w = np.asarray(dense_w, np.float32)
    dense_b = np.asarray(dense_b, np.float32)
    w_ih = np.asarray(w_ih, np.float32)
    w_hh = np.asarray(w_hh, np.float32)
    b_ih = np.asarray(b_ih, np.float32)
    b_hh = np.asarray(b_hh, np.float32)

    nc = _build_nc()
    in_maps = _pack_inputs(t, h0, c0, dense_w, dense_b, w_ih, w_hh, b_ih, b_hh)
    res = bass_utils.run_bass_kernel_spmd(nc, in_maps, core_ids=list(range(N_CORES)))

    h2 = np.concatenate([res.results[c]["h2"] for c in range(N_CORES)], axis=0)
    out = np.zeros((B, 1, 2 * H), np.float32)
    out[:, 0, :H] = h2
    return out
